# revision 1
# baseline (speedup 1.0000x reference)
"""Trainium2 Bass kernel for nn_CrossViewSwapAttention.

Sharding: 8 cores; core c handles batch b=c//4 and BEV window-rows
X in {2*(c%4), 2*(c%4)+1} (16 of the 64 windows). Zero inter-core
communication; each core computes conv/embed/attention for exactly the
feature/BEV pixels its windows need (feature px gathered twice: once in
stage-1 local-window order, once in stage-2 grid-window order).

Host does layout (window-ordered gathers) and weight folds (LN gains
folded into projection weights; LN mean/sigma corrections are rank-1
matmul accumulates). V is fully LN-normalized at projection time (ACT
copy-scale by 1/sigma_v; denominator columns pre-set to 1), so exp()
carries only the per-kv-token r_k*scale operand. Tail kv-chunks (8 of
264) of window pairs share one psum at partition bases {0,64} so the
tail exp runs once per head-pair per pair of windows. Engine balance:
ReLU/BN and psum evacuations fused onto ACT, squares/copies on GPSIMD
where SBUF-only, psum banks rotated for pipelining throughout.

PE alignment rules honored: matmul lhsT/rhs share base partition in
{0,32,64}; head 3 (channels 96:128) is produced into separate base-0
tiles by splitting each projection into M=96 and M=32 matmuls; all
rank-1 row operands live in base-0 (1, n) flat tiles (stat rows are
flattened via SBUF->SBUF DMA); per-token column stats are transposed
into window-aligned (128, 3, 32) tiles.
"""
import numpy as np
import ml_dtypes

BF16 = ml_dtypes.bfloat16

B, N = 2, 6
H, W = 64, 64
DIM = 128
DH = 32
EPS = 1e-5
IMG_H, IMG_W = 224, 480
FH, FW = 32, 88

NWIN = 16
KV = 264
QT = 384
PX_HALF = 4224
PX = 8448
QS = 6144
NPX = 1024
SCALE = DH ** -0.5
CHK = [(0, 128), (128, 128), (256, 8)]   # window-local kv chunks

# constant-blob layouts: (name, free width); bf16 blob and f32 blob each
# loaded with ONE DMA so downstream matmuls wait on a single semaphore
BF_LAYOUT = [('WflT', 128), ('WfpT', 128), ('Wq', 256), ('Wk', 256),
             ('Wv', 264), ('Wp', 256), ('Wma', 512), ('Wmb', 512),
             ('idbf', 128), ('bevst', 128), ('rows1', 2056), ('rows2', 256),
             ('geow', 384)]
F32_LAYOUT = [('bn', 4), ('postgb', 2), ('vqcol', 4), ('bbev', 128),
              ('idf', 128)]
BFO = {}
_o = 0
for _n, _w in BF_LAYOUT:
    BFO[_n] = _o
    _o += _w
BF_W = _o
F32O = {}
_o = 0
for _n, _w in F32_LAYOUT:
    F32O[_n] = _o
    _o += _w
F32_W = _o
# rows1 sub-offsets (row 0): suq, suk, suv, bp, suma, vma
R1 = {'suq': 0, 'suk': 256, 'suv': 512, 'bp': 776, 'suma': 1032,
      'vma': 1544}


def _win_list(x2):
    return [(2 * x2 + wx, wy) for wx in range(2) for wy in range(8)]


def _feat_px(x2):
    out = []
    for (X, Y) in _win_list(x2):
        for r in range(4):
            for c in range(11):
                out.append((4 * X + r, 11 * Y + c))
    for (X, Y) in _win_list(x2):
        for r in range(4):
            for c in range(11):
                out.append((X + 8 * r, Y + 8 * c))
    return np.array(out)


def _bev_px(x2):
    out = []
    for (X, Y) in _win_list(x2):
        for r in range(8):
            for c in range(8):
                out.append((8 * X + r, 8 * Y + c))
    return np.array(out)


def host_prep(inputs):
    f = {k: np.asarray(v) for k, v in inputs.items()}
    g = {}
    bf_parts = {}
    f32_parts = {}

    Wq, Wk, suq, suk, vq = [], [], [], [], []
    Wva, suva, vv = [], [], []
    for s in range(2):
        lg, lb = f['attn_ln_g'][s], f['attn_ln_b'][s]
        Wf = [lg[p][:, None] * f['attn_Wqkv'][s, p] for p in range(3)]
        Wq.append(Wf[0]); Wk.append(Wf[1])
        suq.append(Wf[0].sum(0)); suk.append(Wf[1].sum(0))
        vq.append(lb[0] @ f['attn_Wqkv'][s, 0] + f['attn_bqkv'][s, 0])
        vv.append(lb[2] @ f['attn_Wqkv'][s, 2] + f['attn_bqkv'][s, 2])
        Wa = np.zeros((DIM, 132), np.float32)
        sa = np.zeros((132,), np.float32)
        for h in range(4):
            Wa[:, 33 * h:33 * h + 32] = Wf[2][:, 32 * h:32 * h + 32]
            sa[33 * h:33 * h + 32] = Wf[2][:, 32 * h:32 * h + 32].sum(0)
        Wva.append(Wa); suva.append(sa)

    bf_parts['WflT'] = f['W_fl'].T
    bf_parts['WfpT'] = f['W_fp'].T
    bf_parts['Wq'] = np.stack(Wq, 1).reshape(DIM, 256)
    bf_parts['Wk'] = np.stack(Wk, 1).reshape(DIM, 256)
    bf_parts['Wv'] = np.stack(Wva, 1).reshape(DIM, 264)
    bf_parts['Wp'] = np.stack([f['attn_Wp'][0] / 6.0, f['attn_Wp'][1]],
                              1).reshape(DIM, 256)
    bf_parts['Wma'] = np.stack([f['pn_g'][s][:, None] * f['Wma'][s]
                                for s in range(2)], 1).reshape(DIM, 512)
    bf_parts['Wmb'] = np.stack(
        [f['Wmb'][s].reshape(2, 128, 128) for s in range(2)]
    ).transpose(2, 0, 1, 3).reshape(DIM, 512)
    bf_parts['idbf'] = np.eye(128, dtype=np.float32)
    bev_static = np.zeros((DIM, DIM), np.float32)
    for n in range(N):
        bev_static[3 * n:3 * n + 2] = f['W_bev'].T
    bf_parts['bevst'] = bev_static          # rows 0:18 used
    rows1 = np.zeros((DIM, 2056), np.float32)
    rows1[0, R1['suq']:R1['suq'] + 256] = np.concatenate(suq)
    rows1[0, R1['suk']:R1['suk'] + 256] = np.concatenate(suk)
    rows1[0, R1['suv']:R1['suv'] + 264] = np.concatenate(suva)
    rows1[0, R1['bp']:R1['bp'] + 256] = np.concatenate(
        [f['attn_bp'][s] + vv[s] @ f['attn_Wp'][s] for s in range(2)])
    rows1[0, R1['suma']:R1['suma'] + 512] = np.concatenate(
        [(f['pn_g'][s][:, None] * f['Wma'][s]).sum(0) for s in range(2)])
    rows1[0, R1['vma']:R1['vma'] + 512] = np.concatenate(
        [f['pn_b'][s] @ f['Wma'][s] + f['bma'][s] for s in range(2)])
    bf_parts['rows1'] = rows1
    rows2 = np.zeros((DIM, 256), np.float32)
    rows2[0, 0:256] = np.concatenate([f['bmb'][s] for s in range(2)])
    bf_parts['rows2'] = rows2
    geow = np.zeros((4, 384), np.float32)
    geow[:, 0:128] = f['W_img'].T
    geow[:, 128:256] = f['W_cam'].T
    geow[:, 256:384] = (f['W_img'] - f['W_cam']).T
    bf_parts['geow'] = geow

    blob_bf = np.zeros((DIM, BF_W), np.float32)
    for k, a in bf_parts.items():
        blob_bf[:a.shape[0], BFO[k]:BFO[k] + a.shape[1]] = a
    g['blob_bf'] = blob_bf.astype(BF16)

    f32_parts = {}
    f32_parts['bn'] = np.stack([
        f['bn_fl_g'] / np.sqrt(1.0 + EPS), f['bn_fl_b'],
        f['bn_fp_g'] / np.sqrt(1.0 + EPS), f['bn_fp_b']], 1)
    f32_parts['postgb'] = np.stack([f['post_g'], f['post_b']], 1)
    vqc = np.zeros((DIM, 4), np.float32)
    for s in range(2):
        vqc[0:96, 2 * s] = vq[s][0:96]
        vqc[32 * s:32 * s + 32, 2 * s + 1] = vq[s][96:128]
    f32_parts['vqcol'] = vqc
    bb6 = np.zeros((6, DIM), np.float32)
    bb6[0:6] = np.tile(f['b_bev'][None, :], (6, 1))
    f32_parts['bbev'] = bb6
    f32_parts['idf'] = np.eye(128, dtype=np.float32)
    blob_f32 = np.zeros((DIM, F32_W), np.float32)
    for k, a in f32_parts.items():
        blob_f32[:a.shape[0], F32O[k]:F32O[k] + a.shape[1]] = a
    g['blob_f32'] = blob_f32.astype(np.float32)

    xs = np.linspace(0.0, 1.0, FW) * IMG_W
    ys = np.linspace(0.0, 1.0, FH) * IMG_H
    gx, gy = np.meshgrid(xs, ys)
    plane = np.stack([gx, gy, np.ones_like(gx)], 0).astype(np.float32)

    cores = []
    for c in range(8):
        b, x2 = c // 4, c % 4
        d = {}
        fpx = _feat_px(x2)
        ih = fpx[:, 0].reshape(2 * NWIN, 44)
        iw = fpx[:, 1].reshape(2 * NWIN, 44)
        gfeat = f['feature'][b][:, :, ih, iw]            # (6,128,32,44)
        d['feat'] = np.ascontiguousarray(
            gfeat.transpose(1, 2, 0, 3).reshape(DIM, PX)).astype(BF16)
        pl = plane[:, ih, iw]                            # (3,32,44)
        pb = np.zeros((24, PX), np.float32)
        for n in range(N):
            for j in range(3):
                pb[4 * n + j].reshape(32, N, 44)[:, n, :] = pl[j]
            pb[4 * n + 3].reshape(32, N, 44)[:, n, :] = 1.0
        d['plane_bf'] = pb.astype(BF16)
        bpx = _bev_px(x2)
        gr = f['grid'][:2, bpx[:, 0], bpx[:, 1]]
        gb = np.zeros((18, QS), np.float32)
        for n in range(N):
            for j in range(2):
                gb[3 * n + j].reshape(NWIN, N, 64)[:, n, :] = \
                    gr[j].reshape(NWIN, 64)
            gb[3 * n + 2].reshape(NWIN, N, 64)[:, n, :] = 1.0
        d['grid_bf'] = gb.astype(BF16)
        d['xc'] = np.ascontiguousarray(
            f['x'][b][:, bpx[:, 0], bpx[:, 1]], np.float32)
        geo = np.zeros((4, 48), np.float32)
        geo[:, 0:24] = f['E_inv'][b].transpose(1, 0, 2).reshape(4, 24)
        geo[:, 24:30] = f['E_inv'][b][:, :, 3].T
        geo[0:3, 30:48] = f['I_inv'][b].transpose(1, 0, 2).reshape(3, 18)
        d['geo'] = geo.astype(BF16)
        cores.append(d)
    return g, cores


# ---------------------------------------------------------------------------

def build_nc():
    from contextlib import ExitStack
    import concourse.bass as bass
    import concourse.bacc as bacc
    import concourse.tile as tile
    from concourse import mybir

    F32 = mybir.dt.float32
    BF = mybir.dt.bfloat16
    AF = mybir.ActivationFunctionType
    OP = mybir.AluOpType
    AX = mybir.AxisListType

    nc = bacc.Bacc(None, target_bir_lowering=False)

    def din(name, shape, dt=F32):
        return nc.dram_tensor(name, shape, dt, kind="ExternalInput")

    feat_d = din("feat", [DIM, PX], BF)
    plane_d = din("plane_bf", [24, PX], BF)
    grid_d = din("grid_bf", [18, QS], BF)
    xc_d = din("xc", [DIM, NPX])
    geo_d = din("geo", [4, 48], BF)
    blobbf_d = din("blob_bf", [DIM, BF_W], BF)
    blobf32_d = din("blob_f32", [DIM, F32_W])
    out_d = nc.dram_tensor("out", [DIM, NPX], F32, kind="ExternalOutput")

    ctx = ExitStack()
    with tile.TileContext(nc) as tc, ctx:
        cp = ctx.enter_context(tc.tile_pool(name="consts", bufs=1))
        big = ctx.enter_context(tc.tile_pool(name="big", bufs=1))
        wk = ctx.enter_context(tc.tile_pool(name="work", bufs=2))
        wkb = ctx.enter_context(tc.tile_pool(name="workb", bufs=1))
        stat = ctx.enter_context(tc.tile_pool(name="stats", bufs=1))
        pz = ctx.enter_context(tc.tile_pool(name="psum", bufs=1, space="PSUM"))

        # PSUM tags (static 16KB): SpsA/SpsB 2 banks, OpsA/B/C 1 bank, T1 1 bank
        def PS(tag):
            shapes = {"SpsA": [DIM, 1024], "SpsB": [DIM, 1024],
                      "OpsA": [DIM, 512], "OpsB": [DIM, 512],
                      "OpsC": [DIM, 512], "T1": [DIM, 512]}
            return pz.tile(shapes[tag], F32, tag=tag, name=f"ps_{tag}")

        def PSB(tag):
            return pz.tile([DIM, 1024], BF, tag=tag, name=f"psb_{tag}")

        # single-DMA constant blobs
        cbf = cp.tile([DIM, BF_W], BF)
        nc.sync.dma_start(out=cbf[:], in_=blobbf_d[:, :])
        cf = cp.tile([DIM, F32_W], F32)
        nc.sync.dma_start(out=cf[:], in_=blobf32_d[:, :])
        geo = cp.tile([4, 48], BF)
        nc.sync.dma_start(out=geo[:], in_=geo_d[:, :])
        xc = cp.tile([DIM, NPX], F32)
        nc.sync.dma_start(out=xc[:], in_=xc_d[:, :])

        def bfc(name, rows=DIM):
            o, w = BFO[name], dict(BF_LAYOUT)[name]
            return cbf[:rows, o:o + w]

        def f32c(name, rows=DIM):
            o, w = F32O[name], dict(F32_LAYOUT)[name]
            return cf[:rows, o:o + w]

        WflT = bfc('WflT')
        WfpT = bfc('WfpT')
        Wq = bfc('Wq').rearrange("p (s d) -> p s d", s=2)
        Wk = bfc('Wk').rearrange("p (s d) -> p s d", s=2)
        Wv = bfc('Wv').rearrange("p (s d) -> p s d", s=2)
        Wp = bfc('Wp').rearrange("p (s d) -> p s d", s=2)
        Wma = bfc('Wma').rearrange("p (s d) -> p s d", s=2)
        Wmb = bfc('Wmb').rearrange("p (s h d) -> p s h d", s=2, h=2)
        idbf = bfc('idbf')
        r1o = BFO['rows1']
        suq = cbf[0:1, r1o + R1['suq']:r1o + R1['suq'] + 256]
        suk = cbf[0:1, r1o + R1['suk']:r1o + R1['suk'] + 256]
        suv = cbf[0:1, r1o + R1['suv']:r1o + R1['suv'] + 264]
        bp = cbf[0:1, r1o + R1['bp']:r1o + R1['bp'] + 256]
        suma = cbf[0:1, r1o + R1['suma']:r1o + R1['suma'] + 512]
        vma = cbf[0:1, r1o + R1['vma']:r1o + R1['vma'] + 512]
        bmb = cbf[0:1, BFO['rows2']:BFO['rows2'] + 256]
        bn = f32c('bn')
        postgb = f32c('postgb')
        vqcol = f32c('vqcol')
        bbev = f32c('bbev', rows=6)
        idf = f32c('idf')
        gwo = BFO['geow']
        WimgT = cbf[0:4, gwo:gwo + 128]
        WcamT = cbf[0:4, gwo + 128:gwo + 256]
        WuT = cbf[0:4, gwo + 256:gwo + 384]
        Esb = geo[0:4, 0:24]
        Elast = geo[0:4, 24:30]
        Isb = geo[0:3, 30:48]

        ones_r = cp.tile([1, 512], BF)
        nc.vector.memset(ones_r[:], 1.0)
        Z = cp.tile([DIM, 65], BF)
        nc.vector.memset(Z[:], 0.0)
        nc.vector.memset(Z[:, 32:33], 1.0)

        # persistent activations
        val = big.tile([DIM, PX], BF, tag="val")
        Ktm = [big.tile([96, PX_HALF], BF, tag=f"Ktm{s}", name=f"Ktm{s}")
               for s in range(2)]
        # H3 packs all head-3 data; lhsT/rhs of the head-3 QK matmul
        # share base 32*s:
        #   rows 32s:32s+32, cols 0:4224      = Kt3 stage s
        #   rows 0:32,  cols 4224:10368       = Q1 head3 (16 win x 384)
        #   rows 32:64, cols 4224:5248        = Q2 head3
        H3 = big.tile([64, 10368], BF, tag="H3")
        Vwin = big.tile([DIM, 96, 132], BF, tag="Vwin")
        Vtail = big.tile([DIM, 16, 132], BF, tag="Vtail")
        qim = big.tile([DIM, NWIN, QT], BF, tag="og", name="qim")
        Q1m = big.tile([96, NWIN, QT], BF, tag="Q1m")
        q1 = big.tile([DIM, NPX], F32, tag="qg")
        q1b = big.tile([DIM, NPX], BF, tag="qb", name="q1b")
        hbuf = big.tile([DIM, 8, 256], BF, tag="hbuf")
        hT = big.tile([DIM, 2, NPX], BF, tag="hT")

        # stats: rows, window-aligned cols, and two reusable base-0 flats
        nmuv_r = stat.tile([32, KV], BF)
        nmuk_r = stat.tile([32, KV], BF)
        rimg_r = stat.tile([32, KV], BF)
        rks_c = stat.tile([DIM, 3, 32], F32)
        rvc4 = stat.tile([DIM, 16], F32)
        rkt4 = stat.tile([DIM, 16], F32)
        flatA = stat.tile([1, PX], BF)
        flatB = stat.tile([1, QS], BF)

        TT = nc.vector.tensor_tensor
        TS = nc.vector.tensor_scalar
        CPY = nc.vector.tensor_copy
        MM = nc.tensor.matmul

        # =========== S1: tiny folds ===========
        lhsT_img = cp.tile([24, DIM], BF)
        lhsT_bev = cp.tile([18, DIM], BF)
        nc.sync.dma_start(out=lhsT_bev[:],
                          in_=blobbf_d[0:18, BFO['bevst']:BFO['bevst'] + 128])
        ce_ps = PS("OpsA")
        MM(ce_ps[:6, :DIM], Elast[:, :], WcamT[:, :], start=True, stop=True)
        bias6 = cp.tile([6, DIM], BF)
        TT(out=bias6[:, :], in0=bbev[:, :], in1=ce_ps[:6, :DIM], op=OP.subtract)
        # scatter rows 3n+2 via DMA (engine APs must be quadrant-aligned)
        bev_rows = bass.AP(tensor=lhsT_bev[:, :].tensor,
                           offset=lhsT_bev[2:3, :].offset,
                           ap=[[3 * lhsT_bev[:, :].ap[0][0], 6], [1, DIM]])
        nc.sync.dma_start(out=bev_rows, in_=bias6[:, :])
        sc3 = cp.tile([3, DIM], BF)
        stgA = cp.tile([3, DIM], BF)
        stgU = cp.tile([1, DIM], BF)
        for n in range(N):
            m1 = PS("OpsB")
            MM(m1[:3, :DIM], Esb[:, 4 * n:4 * n + 3], WimgT[:, :],
               start=True, stop=True)
            CPY(sc3[:, :], m1[:3, :DIM])
            m2 = PS("OpsC")
            MM(m2[:3, :DIM], Isb[:, 3 * n:3 * n + 3], sc3[:, :],
               start=True, stop=True)
            CPY(stgA[:, :], m2[:3, :DIM])
            nc.sync.dma_start(out=lhsT_img[4 * n:4 * n + 3, :], in_=stgA[:, :])
            m3 = PS("T1")
            MM(m3[:1, :DIM], Esb[:, 4 * n + 3:4 * n + 4], WuT[:, :],
               start=True, stop=True)
            CPY(stgU[:, :], m3[:1, :DIM])
            nc.sync.dma_start(out=lhsT_img[4 * n + 3:4 * n + 4, :],
                              in_=stgU[:, :])

        def plane_chunk(g4):
            """DMA 4 windows of plane_blocked -> (24, 1056) bf tile."""
            t = wk.tile([24, 4 * KV], BF, tag="ioch", name="plch")
            nc.sync.dma_start(out=t[:, :],
                              in_=plane_d[:, 4 * KV * g4:4 * KV * (g4 + 1)])
            return t

        # =========== S2: loop-1: img-embed norm stats ===========
        stE = PS("OpsA")
        for w in range(32):
            if w % 4 == 0:
                pl4 = plane_chunk(w // 4)
            zsel = Z[:, 32 - w:64 - w]
            eps_ = PS("SpsA" if w % 2 == 0 else "SpsB")
            MM(eps_[:, :KV], lhsT_img[:, :], pl4[:, KV * (w % 4):KV * (w % 4 + 1)],
               start=True, stop=True)
            e2 = wk.tile([DIM, KV], BF, tag="wbfE")
            nc.scalar.square(e2[:, :], eps_[:, :KV])
            MM(stE[:32, :KV], zsel, e2[:, :], start=(w == 0), stop=(w == 31),
               skip_group_check=True)

        # rimg rows + flatA
        t_a = stat.tile([32, KV], F32, tag="sA")
        nc.vector.tensor_scalar_max(t_a[:, :], stE[:32, :KV], 1e-24)
        t_b = stat.tile([32, KV], F32, tag="sB")
        nc.scalar.sqrt(t_b[:, :], t_a[:, :])
        t_c = stat.tile([32, KV], F32, tag="sC")
        nc.vector.reciprocal(t_c[:, :], t_b[:, :])
        CPY(rimg_r[:, :], t_c[:, :])
        nc.sync.dma_start(out=flatA[0:1, :], in_=rimg_r[:, :])

        def ln_rows(st1, st2, nmu_r, nmu_flat_ap):
            mu = stat.tile([32, KV], F32, tag="sA", name="mu_t")
            nc.vector.tensor_scalar_mul(mu[:, :], st1[:32, :KV], 1.0 / 128)
            nc.vector.tensor_scalar_mul(nmu_r[:, :], st1[:32, :KV], -1.0 / 128)
            nc.sync.dma_start(out=nmu_flat_ap, in_=nmu_r[:, :])
            var = stat.tile([32, KV], F32, tag="sB", name="var_t")
            TS(out=var[:, :], in0=st2[:32, :KV], scalar1=1.0 / 128,
               scalar2=EPS, op0=OP.mult, op1=OP.add)
            mu2 = stat.tile([32, KV], F32, tag="sC", name="mu2_t")
            TT(out=mu2[:, :], in0=mu[:, :], in1=mu[:, :], op=OP.mult)
            TT(out=var[:, :], in0=var[:, :], in1=mu2[:, :], op=OP.subtract)
            return var

        def var_to_cols(var, col_out):
            for ci, (off, sz) in enumerate(CHK):
                tp = PS("T1")
                MM(tp[:sz, :32], var[:, off:off + sz], idf[:32, :32],
                   start=True, stop=True, is_transpose=True)
                CPY(col_out[:sz, ci, :], tp[:sz, :32])

        # =========== loop-2a: val conv + stats ===========
        stV1 = PS("OpsB")
        stV2 = PS("OpsC")
        for w in range(32):
            o = KV * w
            if w % 4 == 0:
                f4 = wk.tile([DIM, 4 * KV], BF, tag="ioch", name="fch")
                nc.sync.dma_start(out=f4[:, :], in_=feat_d[:, o:o + 4 * KV])
            fch = f4[:, KV * (w % 4):KV * (w % 4 + 1)]
            zsel = Z[:, 32 - w:64 - w]
            yfl = wk.tile([DIM, KV], BF, tag="wbfA")
            nc.scalar.activation(out=yfl[:, :], in_=fch, func=AF.Relu,
                                 scale=bn[:, 0:1], bias=bn[:, 1:2])
            vps = PS("SpsA" if w % 2 == 0 else "SpsB")
            MM(vps[:, :KV], WflT[:, :], yfl[:, :], start=True, stop=True)
            if w % 2 == 0:
                CPY(val[:, o:o + KV], vps[:, :KV])
            else:
                nc.scalar.copy(val[:, o:o + KV], vps[:, :KV])
            v2 = wk.tile([DIM, KV], BF, tag="wbfD")
            TT(out=v2[:, :], in0=val[:, o:o + KV], in1=val[:, o:o + KV],
               op=OP.mult)
            MM(stV1[:32, :KV], zsel, val[:, o:o + KV],
               start=(w == 0), stop=(w == 31), skip_group_check=True)
            MM(stV2[:32, :KV], zsel, v2[:, :], start=(w == 0), stop=(w == 31),
               skip_group_check=True)

        varV = ln_rows(stV1, stV2, nmuv_r, flatA[0:1, :])  # nmuv -> flatA
        vcol = stat.tile([DIM, 3, 32], F32, tag="sCOL")
        var_to_cols(varV, vcol)
        sgvc = stat.tile([DIM, 3, 32], F32)
        nc.scalar.sqrt(sgvc[:, :, :], vcol[:, :, :])
        rvc = stat.tile([DIM, 3, 32], F32, tag="sCOL2")
        nc.vector.reciprocal(rvc[:, :, :], sgvc[:, :, :])
        # tail rv columns, replicated to partition bases 64k (k = w %% 2)
        for k in range(2):
            sref = rvc[0:8, 2, k:k + 1]
            srcb = bass.AP(tensor=sref.tensor, offset=sref.offset,
                           ap=[sref.ap[0], [16, 2], [2, 8]])
            dref = rvc4[64 * k:64 * k + 8, :]
            nc.sync.dma_start(out=dref, in_=srcb)

        # =========== V projections (window-chunked, 4-bank pipelined) =======
        Vw4 = Vwin[:, :, :].rearrange("p c (h d) -> p c h d", h=4)
        nc.vector.memset(Vw4[:, :, :, 32], 1.0)
        Vt4 = Vtail[:, :, :].rearrange("p g (h d) -> p g h d", h=4)
        nc.vector.memset(Vt4[:, :, :, 32], 1.0)
        vtags = ["OpsA", "OpsB", "OpsC", "T1"]
        vit = 0
        for s in range(2):
            for w in range(NWIN):
                sw = NWIN * s + w
                for ci, (off, sz) in enumerate(CHK):
                    go = KV * sw + off
                    vp = PS(vtags[vit % 4])
                    if ci < 2:
                        b0 = 0
                        dst = Vwin[:sz, 3 * sw + ci, :]
                        scl = rvc[0:sz, ci, sw:sw + 1]
                    else:
                        b0 = 64 * (w % 2)
                        gi = 8 * s + w // 2
                        dst = Vtail[b0:b0 + 8, gi, :]
                        scl = rvc4[b0:b0 + 8, gi:gi + 1]
                    MM(vp[b0:b0 + sz, :132], val[:, go:go + sz], Wv[:, s, :],
                       start=True, stop=False)
                    MM(vp[b0:b0 + sz, :132], flatA[0:1, go:go + sz],
                       suv[0:1, 132 * s:132 * s + 132], start=False, stop=True)
                    dstv = dst.rearrange("p (h d) -> p h d", h=4)
                    vpv = vp[b0:b0 + sz, :132].rearrange("p (h d) -> p h d",
                                                         h=4)
                    if vit % 2 == 0:
                        nc.scalar.activation(out=dstv[:, :, 0:32],
                                             in_=vpv[:, :, 0:32],
                                             func=AF.Identity, scale=scl)
                    else:
                        nc.vector.tensor_scalar_mul(dstv[:, :, 0:32],
                                                    vpv[:, :, 0:32], scl)
                    vit += 1

        # =========== loop-2b: key build (key reuses val's slot; nmuv was
        # consumed by the V-projections so flatA is free for rimg again) ====
        key = big.tile([DIM, PX], BF, tag="val", name="key")
        nc.sync.dma_start(out=flatA[0:1, :], in_=rimg_r[:, :])
        stK1 = PS("OpsB")
        stK2 = PS("OpsC")
        for w in range(32):
            o = KV * w
            if w % 4 == 0:
                f4b = wk.tile([DIM, 4 * KV], BF, tag="ioch", name="fchb")
                nc.sync.dma_start(out=f4b[:, :], in_=feat_d[:, o:o + 4 * KV])
                pl4b = plane_chunk(w // 4)
            fch = f4b[:, KV * (w % 4):KV * (w % 4 + 1)]
            zsel = Z[:, 32 - w:64 - w]
            yfp = wk.tile([DIM, KV], BF, tag="wbfA")
            nc.scalar.activation(out=yfp[:, :], in_=fch, func=AF.Relu,
                                 scale=bn[:, 2:3], bias=bn[:, 3:4])
            # r_img pre-scales the plane chunk (24 rows), so the img-embed
            # matmul accumulates straight into the conv psum
            rrep = PS("T1" if w % 2 == 0 else "OpsA")
            MM(rrep[:24, :KV], ones_r[:1, :24], flatA[0:1, o:o + KV],
               start=True, stop=True)
            pls = wk.tile([24, KV], BF, tag="wbfG")
            TT(out=pls[:, :], in0=pl4b[:24, KV * (w % 4):KV * (w % 4 + 1)],
               in1=rrep[:24, :KV], op=OP.mult)
            kc = PS("SpsA" if w % 2 == 0 else "SpsB")
            MM(kc[:, :KV], WfpT[:, :], yfp[:, :], start=True, stop=False)
            MM(kc[:, :KV], lhsT_img[:, :], pls[:, :], start=False, stop=True)
            if w % 2 == 0:
                CPY(key[:, o:o + KV], kc[:, :KV])
            else:
                nc.scalar.copy(key[:, o:o + KV], kc[:, :KV])
            k2 = wk.tile([DIM, KV], BF, tag="wbfD")
            nc.gpsimd.tensor_tensor(out=k2[:, :], in0=key[:, o:o + KV],
                                    in1=key[:, o:o + KV], op=OP.mult)
            MM(stK1[:32, :KV], zsel, key[:, o:o + KV],
               start=(w == 0), stop=(w == 31), skip_group_check=True)
            MM(stK2[:32, :KV], zsel, k2[:, :], start=(w == 0), stop=(w == 31),
               skip_group_check=True)

        varK = ln_rows(stK1, stK2, nmuk_r, flatA[0:1, :])  # nmuk -> flatA
        kcol = stat.tile([DIM, 3, 32], F32, tag="sCOL")
        var_to_cols(varK, kcol)
        ksq = stat.tile([DIM, 3, 32], F32, tag="sCOL2")
        nc.scalar.sqrt(ksq[:, :, :], kcol[:, :, :])
        nc.vector.reciprocal(rks_c[:, :, :], ksq[:, :, :])
        nc.vector.tensor_scalar_mul(rks_c[:, :, :], rks_c[:, :, :], SCALE)
        for k in range(2):
            sref = rks_c[0:8, 2, k:k + 1]
            srcb = bass.AP(tensor=sref.tensor, offset=sref.offset,
                           ap=[sref.ap[0], [16, 2], [2, 8]])
            dref = rkt4[64 * k:64 * k + 8, :]
            nc.sync.dma_start(out=dref, in_=srcb)

        # =========== K projections (pipelined; ACT/GPSIMD evacuate) ========
        kit = 0
        for s in range(2):
            for j in range(9):
                o, cs = 512 * j, min(512, PX_HALF - 512 * j)
                go = PX_HALF * s + o
                kA = PS("SpsA" if kit % 2 == 0 else "OpsA")
                MM(kA[:96, :cs], Wk[:, s, 0:96], key[:, go:go + cs],
                   start=True, stop=False)
                MM(kA[:96, :cs], suk[0:1, 128 * s:128 * s + 96],
                   flatA[0:1, go:go + cs], start=False, stop=True)
                nc.scalar.copy(Ktm[s][:, o:o + cs], kA[:96, :cs])
                k3 = PS("SpsB" if kit % 2 == 0 else "OpsB")
                MM(k3[32 * s:32 * s + 32, :cs], Wk[:, s, 96:128],
                   key[:, go:go + cs], start=True, stop=False)
                MM(k3[32 * s:32 * s + 32, :cs],
                   suk[0:1, 128 * s + 96:128 * s + 128],
                   flatA[0:1, go:go + cs], start=False, stop=True)
                CPY(H3[32 * s:32 * s + 32, o:o + cs],
                    k3[32 * s:32 * s + 32, :cs])
                kit += 1

        # =========== S7: q-side ===========
        def grid_chunk(g4):
            t = wk.tile([18, 4 * QT], BF, tag="ioch", name="grch")
            nc.sync.dma_start(out=t[:, :],
                              in_=grid_d[:, 4 * QT * g4:4 * QT * (g4 + 1)])
            return t

        stQe = PS("OpsA")
        for w in range(NWIN):
            if w % 4 == 0:
                g4t = grid_chunk(w // 4)
            eq = PS("SpsA" if w % 2 == 0 else "SpsB")
            MM(eq[:, :QT], lhsT_bev[:, :], g4t[:, QT * (w % 4):QT * (w % 4 + 1)],
               start=True, stop=True)
            e2 = wk.tile([DIM, QT], BF, tag="wbfA")
            nc.scalar.square(e2[:, :], eq[:, :QT])
            MM(stQe[:16, :QT], Z[:, 32 - w:48 - w], e2[:, :],
               start=(w == 0), stop=(w == 15), skip_group_check=True)
        t_a2 = stat.tile([NWIN, QT], F32, tag="sA")
        nc.vector.tensor_scalar_max(t_a2[:, :], stQe[:16, :QT], 1e-24)
        t_b2 = stat.tile([NWIN, QT], F32, tag="sB")
        nc.scalar.sqrt(t_b2[:, :], t_a2[:, :])
        t_c2 = stat.tile([NWIN, QT], F32, tag="sC")
        nc.vector.reciprocal(t_c2[:, :], t_b2[:, :])
        rbev_r = stat.tile([NWIN, QT], BF, tag="sE")
        CPY(rbev_r[:, :], t_c2[:, :])
        nc.sync.dma_start(out=flatB[0:1, :QS], in_=rbev_r[:, :])  # rbev->B

        stQ1 = PS("OpsB")
        stQ2 = PS("OpsC")
        for w in range(NWIN):
            if w % 4 == 0:
                g4t = grid_chunk(w // 4)
            # rbev pre-scales the grid chunk (18 rows): eq = normalized embed
            rrep = PS("T1" if w % 2 == 0 else "OpsA")
            MM(rrep[:18, :QT], ones_r[:1, :18],
               flatB[0:1, QT * w:QT * w + QT], start=True, stop=True)
            gsc = wk.tile([18, QT], BF, tag="wbfG")
            TT(out=gsc[:, :], in0=g4t[:18, QT * (w % 4):QT * (w % 4 + 1)],
               in1=rrep[:18, :QT], op=OP.mult)
            eq = PS("SpsA" if w % 2 == 0 else "SpsB")
            MM(eq[:, :QT], lhsT_bev[:, :], gsc[:, :], start=True, stop=True)
            # qim = eq + x (x broadcast across the 6 cameras via 0-stride AP)
            xs_ = xc[:, 64 * w:64 * w + 64]
            xb = bass.AP(tensor=xs_.tensor, offset=xs_.offset,
                         ap=[xs_.ap[0], [0, N], [1, 64]])
            qv = qim[:, w, :].rearrange("p (n q) -> p n q", n=N)
            ev = eq[:, :QT].rearrange("p (n q) -> p n q", n=N)
            TT(out=qv, in0=ev, in1=xb, op=OP.add)
            q2t = wk.tile([DIM, QT], BF, tag="wbfA")
            nc.gpsimd.tensor_tensor(out=q2t[:, :], in0=qim[:, w, :],
                                    in1=qim[:, w, :], op=OP.mult)
            MM(stQ1[:16, :QT], Z[:, 32 - w:48 - w], qim[:, w, :],
               start=(w == 0), stop=(w == 15), skip_group_check=True)
            MM(stQ2[:16, :QT], Z[:, 32 - w:48 - w], q2t[:, :],
               start=(w == 0), stop=(w == 15), skip_group_check=True)

        # q stats: (-mu*r) row -> flatA, rq -> flatB (rbev consumed)
        muq = stat.tile([NWIN, QT], F32, tag="sA")
        nc.vector.tensor_scalar_mul(muq[:, :], stQ1[:16, :QT], 1.0 / 128)
        varq = stat.tile([NWIN, QT], F32, tag="sB")
        TS(out=varq[:, :], in0=stQ2[:16, :QT], scalar1=1.0 / 128, scalar2=EPS,
           op0=OP.mult, op1=OP.add)
        mu2q = stat.tile([NWIN, QT], F32, tag="sC")
        TT(out=mu2q[:, :], in0=muq[:, :], in1=muq[:, :], op=OP.mult)
        TT(out=varq[:, :], in0=varq[:, :], in1=mu2q[:, :], op=OP.subtract)
        sgq = stat.tile([NWIN, QT], F32, tag="sD")
        nc.scalar.sqrt(sgq[:, :], varq[:, :])
        rqt = stat.tile([NWIN, QT], F32, tag="sJ", name="rqt")
        nc.vector.reciprocal(rqt[:, :], sgq[:, :])
        rq_r = stat.tile([NWIN, QT], BF, tag="sE", name="rq_r")
        CPY(rq_r[:, :], rqt[:, :])
        nc.sync.dma_start(out=flatB[0:1, :QS], in_=rq_r[:, :])
        nmur_f = stat.tile([NWIN, QT], F32, tag="sC", name="nmur_f")
        nc.vector.scalar_tensor_tensor(out=nmur_f[:, :], in0=muq[:, :],
                                       scalar=-1.0, in1=rqt[:, :],
                                       op0=OP.mult, op1=OP.mult)
        nmur_b = stat.tile([NWIN, QT], BF, tag="sE", name="nmur_b")
        CPY(nmur_b[:, :], nmur_f[:, :])
        nc.sync.dma_start(out=flatA[0:1, :QS], in_=nmur_b[:, :])

        def q_project(s, rhs_bf, ncols, st0, Qm_out, Q3_out, r_flat,
                      nmur_flat, foff=0, pp=0):
            # qs = rhs * r (per-token LN scale applied up front); the rank-1
            # mean term then uses the (-mu*r) row, and the evacuations just
            # add the LN-bias projection column.
            rrep = PS("T1" if pp % 2 == 0 else "OpsA")
            MM(rrep[:, :ncols], ones_r[:1, :DIM],
               r_flat[0:1, st0:st0 + ncols], start=True, stop=True)
            qs = wk.tile([DIM, QT], BF, tag="wbfG", name="qs")
            TT(out=qs[:, :ncols], in0=rhs_bf, in1=rrep[:, :ncols], op=OP.mult)
            qA = PS("SpsA" if pp % 2 == 0 else "SpsB")
            MM(qA[:96, :ncols], Wq[:, s, 0:96], qs[:, :ncols],
               start=True, stop=False)
            MM(qA[:96, :ncols], suq[0:1, 128 * s:128 * s + 96],
               nmur_flat[0:1, foff + st0:foff + st0 + ncols],
               start=False, stop=True)
            r0 = 32 * s
            q3 = PS("OpsB" if pp % 2 == 0 else "OpsC")
            MM(q3[r0:r0 + 32, :ncols], Wq[:, s, 96:128], qs[:, :ncols],
               start=True, stop=False)
            MM(q3[r0:r0 + 32, :ncols], suq[0:1, 128 * s + 96:128 * s + 128],
               nmur_flat[0:1, foff + st0:foff + st0 + ncols],
               start=False, stop=True)
            nc.scalar.activation(out=Qm_out, in_=qA[:96, :ncols],
                                 func=AF.Identity,
                                 bias=vqcol[0:96, 2 * s:2 * s + 1])
            nc.vector.tensor_scalar_add(
                Q3_out, q3[r0:r0 + 32, :ncols],
                vqcol[r0:r0 + 32, 2 * s + 1:2 * s + 2])

        for w in range(NWIN):
            q_project(0, qim[:, w, :], QT, QT * w, Q1m[:, w, :],
                      H3[0:32, 4224 + QT * w:4224 + QT * w + QT],
                      flatB, flatA, pp=w)

        # =========== attention ===========
        def attn_tails(s, t, nq, qmf, q3f, Ptl):
            # tails (8 kv) of windows 2t..2t+1 share one psum at partition
            # bases 64k, so the tail exp runs once per head-pair per group
            gi = 8 * s + t
            for g2 in range(2):
                TSp = PS("SpsA" if g2 == 0 else "SpsB")
                for k in range(2):
                    w = 2 * t + k
                    bk = 64 * k
                    for hh in range(2):
                        h = 2 * g2 + hh
                        if h < 3:
                            lhs = Ktm[s][32 * h:32 * h + 32,
                                         KV * w + 256:KV * w + 264]
                            rhs = qmf(w, h)
                        else:
                            lhs = H3[32 * s:32 * s + 32,
                                     KV * w + 256:KV * w + 264]
                            rhs = q3f(w)
                        MM(TSp[bk:bk + 8, 512 * hh:512 * hh + nq], lhs, rhs,
                           start=True, stop=True, skip_group_check=True)
                sin = TSp[:, :].rearrange("p (h f) -> p h f", h=2)
                nc.scalar.activation(out=Ptl[:, 2 * g2:2 * g2 + 2, :],
                                     in_=sin[:, :, :nq], func=AF.Exp,
                                     scale=rkt4[:, gi:gi + 1])

        def attention(s, w, nq, qm, q3ap, oT_out, pt_tag, Ptl):
            Pt = wkb.tile([DIM, 2, 4, nq], BF, tag=pt_tag, name=pt_tag)
            sw = NWIN * s + w
            gi = 8 * s + w // 2
            bk = 64 * (w % 2)
            for ci, (off, sz) in enumerate(CHK[:2]):
                for g2 in range(2):
                    Sps = PS("SpsA" if (2 * ci + g2) % 2 == 0 else "SpsB")
                    for hh in range(2):
                        h = 2 * g2 + hh
                        if h < 3:
                            lhs = Ktm[s][32 * h:32 * h + 32,
                                         KV * w + off:KV * w + off + sz]
                            rhs = qm(h)
                        else:
                            lhs = H3[32 * s:32 * s + 32,
                                     KV * w + off:KV * w + off + sz]
                            rhs = q3ap
                        MM(Sps[:sz, 512 * hh:512 * hh + nq], lhs, rhs,
                           start=True, stop=True)
                    sin = Sps[:sz, :].rearrange("p (h f) -> p h f", h=2)
                    nc.scalar.activation(
                        out=Pt[:sz, ci, 2 * g2:2 * g2 + 2, :],
                        in_=sin[:, :, :nq], func=AF.Exp,
                        scale=rks_c[:sz, ci, sw:sw + 1])
            nqt = (nq + 127) // 128
            qtw = min(128, nq)
            Ops = [PS(t) for t in ("OpsA", "OpsB", "OpsC")[:nqt]]
            for qt in range(nqt):
                for h in range(4):
                    for ci, (off, sz) in enumerate(CHK[:2]):
                        MM(Ops[qt][:qtw, 33 * h:33 * h + 33],
                           Pt[:sz, ci, h, 128 * qt:128 * qt + qtw],
                           Vwin[:sz, 3 * sw + ci, 33 * h:33 * h + 33],
                           start=(ci == 0), stop=False)
                    MM(Ops[qt][:qtw, 33 * h:33 * h + 33],
                       Ptl[bk:bk + 8, h, 128 * qt:128 * qt + qtw],
                       Vtail[bk:bk + 8, gi, 33 * h:33 * h + 33],
                       start=False, stop=True)
            osc = wk.tile([DIM, 3, DIM], BF, tag="osc", name=f"osc{pt_tag}")
            for qt in range(nqt):
                rs = wk.tile([DIM, 4], F32, tag="rsums")
                ov = Ops[qt][:qtw, :132].rearrange("p (h d) -> p h d", h=4)
                nc.vector.reciprocal(rs[:qtw, :], ov[:, :, 32])
                rsq = rs[:qtw, :]
                rsb4 = bass.AP(tensor=rs.tensor, offset=rsq.offset,
                               ap=[rsq.ap[0], rsq.ap[1], [0, 32]])
                oscv = osc[:qtw, qt, :].rearrange("p (h d) -> p h d", h=4)
                TT(out=oscv, in0=ov[:, :, 0:32], in1=rsb4, op=OP.mult)
            for qt in range(nqt):
                tp = PSB("T1" if qt % 2 == 0 else "OpsA")
                MM(tp[:, :qtw], osc[:qtw, qt, :], idbf[:qtw, :qtw],
                   start=True, stop=True, is_transpose=True)
                CPY(oT_out[:, 128 * qt:128 * qt + qtw], tp[:, :qtw])

        o1 = big.tile([DIM, NPX], F32, tag="og", name="o1")
        qmf1 = lambda w, h: Q1m[32 * h:32 * h + 32, w, :]
        q3f1 = lambda w: H3[0:32, 4224 + QT * w:4224 + QT * w + QT]
        for w in range(NWIN):
            if w % 2 == 0:
                Ptl1 = wkb.tile([DIM, 4, QT], BF, tag="Ptl", bufs=1,
                                name="Ptl1")
                attn_tails(0, w // 2, QT, qmf1, q3f1, Ptl1)
            oT = wk.tile([DIM, QT], BF, tag="oT1")
            attention(0, w, QT, lambda h: qmf1(w, h), q3f1(w),
                      oT, "Pt1", Ptl1)
            mred = wk.tile([DIM, 64], F32, tag="wf32c", name="mred")
            oTv = bass.AP(tensor=oT[:, :].tensor, offset=oT[:, :].offset,
                          ap=[oT[:, :].ap[0], [1, 64], [64, N]])
            nc.vector.tensor_reduce(out=mred[:, :], in_=oTv, axis=AX.X,
                                    op=OP.add)
            mb = wk.tile([DIM, 64], BF, tag="wbfF")
            nc.gpsimd.tensor_copy(mb[:, :], mred[:, :])
            zps = PS("T1")
            MM(zps[:, :64], Wp[:, 0, :], mb[:, :], start=True, stop=False)
            MM(zps[:, :64], bp[0:1, 0:128], ones_r[:1, :64],
               start=False, stop=True)
            TT(out=o1[:, 64 * w:64 * w + 64], in0=zps[:, :64],
               in1=xc[:, 64 * w:64 * w + 64], op=OP.add)

        # =========== MLP ===========
        def stats8(src_bf, nmu_fap, sg_fap, r_fap, rc_col,
                   nmu_times_r=False):
            st1 = PS("OpsA")
            st2 = PS("OpsB")
            asq = wkb.tile([DIM, NPX], BF, tag="Pt1", name="asq")
            TT(out=asq[:, :], in0=src_bf[:, :], in1=src_bf[:, :], op=OP.mult)
            for j in range(8):
                MM(st1[:8, :128], Z[:, 32 - j:40 - j],
                   src_bf[:, 128 * j:128 * j + 128],
                   start=(j == 0), stop=(j == 7), skip_group_check=True)
                MM(st2[:8, :128], Z[:, 32 - j:40 - j],
                   asq[:, 128 * j:128 * j + 128],
                   start=(j == 0), stop=(j == 7), skip_group_check=True)
            mu = stat.tile([8, 128], F32, tag="sA", name="mu8")
            nc.vector.tensor_scalar_mul(mu[:, :], st1[:8, :128], 1.0 / 128)
            if nmu_fap is not None and not nmu_times_r:
                nmu8 = stat.tile([8, 128], BF, tag="sE", name="nmu8")
                nc.vector.tensor_scalar_mul(nmu8[:, :], st1[:8, :128],
                                            -1.0 / 128)
                nc.sync.dma_start(out=nmu_fap, in_=nmu8[:, :])
            var = stat.tile([8, 128], F32, tag="sB", name="var8")
            TS(out=var[:, :], in0=st2[:8, :128], scalar1=1.0 / 128,
               scalar2=EPS, op0=OP.mult, op1=OP.add)
            mu2 = stat.tile([8, 128], F32, tag="sC", name="mu28")
            TT(out=mu2[:, :], in0=mu[:, :], in1=mu[:, :], op=OP.mult)
            TT(out=var[:, :], in0=var[:, :], in1=mu2[:, :], op=OP.subtract)
            sgf = stat.tile([8, 128], F32, tag="sD", name="sg8")
            nc.scalar.sqrt(sgf[:, :], var[:, :])
            if sg_fap is not None:
                sg8 = stat.tile([8, 128], BF, tag="sE", name="sg8b")
                CPY(sg8[:, :], sgf[:, :])
                nc.sync.dma_start(out=sg_fap, in_=sg8[:, :])
            if rc_col is not None:
                tp2 = PS("T1")
                MM(tp2[:, :8], sgf[:, :], idf[:8, :8], start=True, stop=True,
                   is_transpose=True)
                sgc = stat.tile([DIM, 8], F32, tag="sF", name="sgc")
                CPY(sgc[:, :], tp2[:, :8])
                nc.vector.reciprocal(rc_col[:, :], sgc[:, :])
            if r_fap is not None:
                rf = stat.tile([8, 128], F32, tag="sJ", name="r8")
                nc.vector.reciprocal(rf[:, :], sgf[:, :])
                r8 = stat.tile([8, 128], BF, tag="sE", name="r8b")
                CPY(r8[:, :], rf[:, :])
                nc.sync.dma_start(out=r_fap, in_=r8[:, :])
                if nmu_times_r:
                    nm8f = stat.tile([8, 128], F32, tag="sC", name="nm8f")
                    nc.vector.scalar_tensor_tensor(
                        out=nm8f[:, :], in0=mu[:, :], scalar=-1.0,
                        in1=rf[:, :], op0=OP.mult, op1=OP.mult)
                    nm8b = stat.tile([8, 128], BF, tag="sE", name="nm8b")
                    CPY(nm8b[:, :], nm8f[:, :])
                    nc.sync.dma_start(out=nmu_fap, in_=nm8b[:, :])
            return sgf, mu

        rcM = stat.tile([DIM, 8], F32)

        def mlp(s, a_f32, out_f32):
            ab = big.tile([DIM, NPX], BF, tag="qb", name=f"mlpin{s}")
            CPY(ab[:, :], a_f32[:, :])
            stats8(ab, flatA[0:1, 0:1024], flatA[0:1, 1024:2048], None, rcM)
            for j in range(8):
                hp = PS("SpsA")
                MM(hp[:, :256], ab[:, 128 * j:128 * j + 128], Wma[:, s, :],
                   start=True, stop=False)
                MM(hp[:, :256], flatA[0:1, 128 * j:128 * j + 128],
                   suma[0:1, 256 * s:256 * s + 256], start=False, stop=False)
                MM(hp[:, :256], flatA[0:1, 1024 + 128 * j:1152 + 128 * j],
                   vma[0:1, 256 * s:256 * s + 256], start=False, stop=True)
                nc.scalar.activation(out=hbuf[:, j, :], in_=hp[:, :256],
                                     func=AF.Gelu, scale=rcM[:, j:j + 1])
            for j in range(8):
                for hf in range(2):
                    tp2 = PSB("T1" if (2 * j + hf) % 2 == 0 else "OpsC")
                    MM(tp2[:, :DIM], hbuf[:, j, 128 * hf:128 * hf + 128],
                       idbf[:, :], start=True, stop=True, is_transpose=True)
                    if hf == 0:
                        nc.scalar.copy(hT[:, hf, 128 * j:128 * j + 128],
                                       tp2[:, :DIM])
                    else:
                        CPY(hT[:, hf, 128 * j:128 * j + 128], tp2[:, :DIM])
            for jc in range(2):
                yp = PS("SpsB")
                MM(yp[:, :512], Wmb[:, s, 0, :],
                   hT[:, 0, 512 * jc:512 * jc + 512], start=True, stop=False)
                MM(yp[:, :512], Wmb[:, s, 1, :],
                   hT[:, 1, 512 * jc:512 * jc + 512], start=False, stop=False)
                MM(yp[:, :512], bmb[0:1, 128 * s:128 * s + 128],
                   ones_r[:1, :512], start=False, stop=True)
                TT(out=out_f32[:, 512 * jc:512 * jc + 512], in0=yp[:, :512],
                   in1=a_f32[:, 512 * jc:512 * jc + 512], op=OP.add)

        mlp(0, o1, q1)
        CPY(q1b[:, :], q1[:, :])

        # =========== stage-2 Q ===========
        stats8(q1b, flatA[0:1, 2048:3072], None, flatB[0:1, :1024], None,
               nmu_times_r=True)
        Q2m = big.tile([96, NPX], BF, tag="Q1m", name="Q2m")
        for j in range(8):
            q_project(1, q1b[:, 128 * j:128 * j + 128], 128, 128 * j,
                      Q2m[:, 128 * j:128 * j + 128],
                      H3[32:64, 4224 + 128 * j:4224 + 128 * j + 128],
                      flatB, flatA, foff=2048, pp=j)

        o2 = big.tile([DIM, NPX], F32, tag="og", name="o2")
        qmf2 = lambda w, h: Q2m[32 * h:32 * h + 32, 64 * w:64 * w + 64]
        q3f2 = lambda w: H3[32:64, 4224 + 64 * w:4224 + 64 * w + 64]
        for w in range(NWIN):
            if w % 2 == 0:
                Ptl2 = wkb.tile([DIM, 4, 64], BF, tag="Ptl2", bufs=1,
                                name="Ptl2")
                attn_tails(1, w // 2, 64, qmf2, q3f2, Ptl2)
            oT = wk.tile([DIM, 64], BF, tag="oT2")
            attention(1, w, 64, lambda h: qmf2(w, h), q3f2(w),
                      oT, "Pt2", Ptl2)
            zps = PS("T1")
            MM(zps[:, :64], Wp[:, 1, :], oT[:, :], start=True, stop=False)
            MM(zps[:, :64], bp[0:1, 128:256], ones_r[:1, :64],
               start=False, stop=True)
            TT(out=o2[:, 64 * w:64 * w + 64], in0=zps[:, :64],
               in1=q1[:, 64 * w:64 * w + 64], op=OP.add)

        q2 = big.tile([DIM, NPX], F32, tag="qg", name="q2")
        mlp(1, o2, q2)

        # =========== post-LN (transpose-apply) ===========
        q2b = big.tile([DIM, NPX], BF, tag="qb", name="q2b")
        CPY(q2b[:, :], q2[:, :])
        sgP, muP = stats8(q2b, None, None, None, None)
        # r3 col, -r3*mu col
        r3 = stat.tile([8, 128], F32, tag="sG", name="r3")
        nc.vector.reciprocal(r3[:, :], sgP[:, :])
        nrmu = stat.tile([8, 128], F32, tag="sH", name="nrmu")
        TT(out=nrmu[:, :], in0=r3[:, :], in1=muP[:, :], op=OP.mult)
        nc.vector.tensor_scalar_mul(nrmu[:, :], nrmu[:, :], -1.0)
        tpr = PS("T1")
        MM(tpr[:, :8], r3[:, :], idf[:8, :8], start=True, stop=True,
           is_transpose=True)
        r3c = stat.tile([DIM, 8], F32, tag="sF", name="r3c")
        CPY(r3c[:, :], tpr[:, :8])
        tpm = PS("T1")
        MM(tpm[:, :8], nrmu[:, :], idf[:8, :8], start=True, stop=True,
           is_transpose=True)
        nrmuc = stat.tile([DIM, 8], F32, tag="sI", name="nrmuc")
        CPY(nrmuc[:, :], tpm[:, :8])
        outsb = big.tile([DIM, NPX], F32, tag="og", name="outsb")
        for j in range(8):
            tq2 = PS("SpsA")
            MM(tq2[:, :DIM], q2[:, 128 * j:128 * j + 128], idf[:, :],
               start=True, stop=True, is_transpose=True)
            aT = wk.tile([DIM, DIM], F32, tag="wf32b", name="aT")
            TS(out=aT[:, :], in0=tq2[:, :DIM], scalar1=r3c[:, j:j + 1],
               scalar2=nrmuc[:, j:j + 1], op0=OP.mult, op1=OP.add)
            tq3 = PS("SpsB")
            MM(tq3[:, :DIM], aT[:, :], idf[:, :], start=True, stop=True,
               is_transpose=True)
            TS(out=outsb[:, 128 * j:128 * j + 128], in0=tq3[:, :DIM],
               scalar1=postgb[:, 0:1], scalar2=postgb[:, 1:2],
               op0=OP.mult, op1=OP.add)
        nc.sync.dma_start(out=out_d[:, :], in_=outsb[:, :])

    nc.compile()
    return nc


_NC_CACHE = None

WNAMES = ['blob_bf', 'blob_f32']
CNAMES = ['feat', 'plane_bf', 'grid_bf', 'xc', 'geo']


def make_in_maps(g, cores):
    in_maps = []
    for c in range(8):
        m = {k: cores[c][k] for k in CNAMES}
        m.update({k: g[k] for k in WNAMES})
        in_maps.append(m)
    return in_maps


def kernel(**inputs):
    global _NC_CACHE
    g, cores = host_prep(inputs)
    if _NC_CACHE is None:
        _NC_CACHE = build_nc()
    nc = _NC_CACHE
    in_maps = make_in_maps(g, cores)
    from concourse.bass_utils import run_bass_kernel_spmd
    res = run_bass_kernel_spmd(nc, in_maps, core_ids=list(range(8)))
    out = np.zeros((B, DIM, H, W), np.float32)
    for c in range(8):
        b, x2 = c // 4, c % 4
        oc = np.asarray(res.results[c]['out'])
        bpx = _bev_px(x2)
        out[b][:, bpx[:, 0], bpx[:, 1]] = oc
    return out

